# revision 15
# baseline (speedup 1.0000x reference)
"""Trainium2 Bass kernel for nn_Attn_69776038691596.

reference computes:
    proj     = einsum('bsh,kh->bsk', enc, W) + bias          # (B,S,H)
    energies = einsum('bh,bsh->bs', hid, proj)               # (B,S)
    out      = softmax(energies, axis=0)                     # over batch

Algebraic rewrite (exact in real arithmetic):
    u[b,:] = hid[b,:] @ W          # (B,H)  -- tiny matmul
    c[b]   = hid[b,:] . bias       # (B,)
    energies[b,s] = enc[b,s,:] . u[b,:] + c[b]

This turns a 275-GFLOP matmul into a 0.27-GFLOP weighted reduction that is
bound by reading encoder_output (512 MB) from HBM once: 64 MiB/core at
~358 GB/s => ~188 us hard floor.

Sharding: split the S axis (2048 -> 8 x 256) across the 8 cores. The softmax
runs over the batch axis, which every core holds entirely, so no collectives
are needed.

Per-core schedule:
  - enc streams on the dedicated qActDynamicHW ring (nc.scalar) as 2 MB
    paired-b DMAs (8 KB contiguous per partition descriptor); W/hid/bias/out
    use the qSPDynamicHW ring (nc.sync); the tiny per-b stg staging DMAs use
    SWDGE (nc.gpsimd). A waiting DMA head-of-line-blocks only its own ring,
    so enc never stalls behind W/stg waits.
  - phase 0: u = hid @ W on PE (fp32, accumulated over 8 k-chunks in PSUM);
    u split into 2 exact bf16 terms (hi+mid carries ~17 mantissa bits);
    c_row = bias . hidT on PE; cbF[s,b] = c[b] broadcast via a K=1
    ones-matmul (used as the reduction's initial value).
  - main loop over b-pairs: one 2 MB DMA streams enc[2b:2b+2] into a
    (128, 2, 2, H) tile (partition p = s//2); u[b] is broadcast into PSUM by
    a K=2 bf16 ones-matmul; ONE DVE tensor_tensor_reduce per (b, r) does
    multiply + h-sum + c[b] offset in a single pass, writing E[s%2][s//2, b]
    directly (ScalarE is entirely free; DVE ~2.3us/b < DMA 2.9us/b).
  - softmax over the free (b) axis of each Er; PE-transpose the (128, 64b)
    results, DVE-interleave r, one output DMA.
"""
import sys

sys.path.insert(0, "/opt/trn_rl_repo")

import numpy as np

B, S, H = 64, 2048, 1024
N_CORES = 8
S_LOC = S // N_CORES  # 256

_CACHE = {}


def build_nc(s_loc=S_LOC, enc_ring=None, use_ttr=None):
    """Build + compile the per-core Bass module. s_loc must be divisible by 256."""
    import os
    if enc_ring is None:
        enc_ring = os.environ.get("K_ENC_RING", "sync")
    if use_ttr is None:
        use_ttr = int(os.environ.get("K_USE_TTR", "0"))
    small_ring = os.environ.get("K_SMALL_RING", "sync")
    import concourse.bass as bass
    import concourse.bacc as bacc
    import concourse.tile as tile
    from concourse import mybir
    from concourse.masks import make_identity
    from contextlib import ExitStack

    f32 = mybir.dt.float32
    bf16 = mybir.dt.bfloat16
    Alu = mybir.AluOpType
    Act = mybir.ActivationFunctionType
    X = mybir.AxisListType.X

    nc = bacc.Bacc("TRN2", target_bir_lowering=False, debug=False,
                   num_devices=N_CORES)
    enc = nc.dram_tensor("enc", [B, s_loc, H], f32, kind="ExternalInput").ap()
    hid = nc.dram_tensor("hid", [B, H], f32, kind="ExternalInput").ap()
    W = nc.dram_tensor("W", [H, H], f32, kind="ExternalInput").ap()
    bias = nc.dram_tensor("bias", [1, H], f32, kind="ExternalInput").ap()
    out = nc.dram_tensor("out", [B, s_loc], f32, kind="ExternalOutput").ap()

    pp = s_loc // 2  # 128 partitions, s = 2p + r

    with ExitStack() as ctx:
        tc = ctx.enter_context(tile.TileContext(nc))
        singles = ctx.enter_context(tc.tile_pool(name="singles", bufs=1))
        wpool = ctx.enter_context(tc.tile_pool(name="wpool", bufs=2))
        chunks = ctx.enter_context(tc.tile_pool(name="chunks", bufs=9))
        small = ctx.enter_context(tc.tile_pool(name="small", bufs=1))
        psum = ctx.enter_context(tc.tile_pool(name="psum", bufs=2, space="PSUM"))
        psum1 = ctx.enter_context(tc.tile_pool(name="psum1", bufs=1, space="PSUM"))
        psumB = ctx.enter_context(tc.tile_pool(name="psumB", bufs=2, space="PSUM"))
        stgpool = ctx.enter_context(tc.tile_pool(name="stgpool", bufs=8))

        # ---------- enc stream: dedicated scalar (ACT) HWDGE ring ----------
        # paired-b tiles: partition p = s//2, free (b2, r, h); descriptors are
        # 8 KB contiguous per (partition, b2).
        enc_eng = nc.scalar if enc_ring == "scalar" else nc.sync
        sml_eng = nc.scalar if small_ring == "scalar" else nc.sync
        enc1 = enc.rearrange("b (p r) h -> b p (r h)", r=2)
        NPAIR = B // 2 - 1  # last 2 b's go as singles to shorten the tail
        NCHUNK = NPAIR + 2
        cks = []

        def issue_ck(i):
            """Issue the i-th enc chunk DMA (pairs, then two singles)."""
            if i < NPAIR:
                ck = chunks.tile([pp, 2, 2, H], f32, tag="ck", name=f"ck{i}")
                pair_ap = bass.AP(
                    tensor=enc.tensor,
                    offset=enc.offset + i * 2 * s_loc * H,
                    ap=[[2 * H, pp], [s_loc * H, 2], [1, 2 * H]])
                enc_eng.dma_start(out=ck, in_=pair_ap)
            else:
                ck = chunks.tile([pp, 1, 2, H], f32, tag="ck", name=f"ck{i}")
                enc_eng.dma_start(out=ck[:, 0, :, :],
                                  in_=enc1[2 * NPAIR + (i - NPAIR)])
            cks.append(ck)

        # Prefetch fewer chunks than the pool holds so an enc DMA never waits
        # on a slot freed by compute that sits behind it in its own FIFO ring.
        PF = 7
        for i in range(PF):
            issue_ck(i)

        # ---------- phase 0 (sync ring + PE/DVE) ----------
        ident64 = singles.tile([64, 64], f32, tag="ident64")
        make_identity(nc, ident64)
        ident128 = singles.tile([128, 128], f32, tag="ident128")
        make_identity(nc, ident128)
        ones1 = singles.tile([1, 128], f32, tag="ones1")
        nc.vector.memset(ones1, 1.0)
        ones2 = singles.tile([2, 128], bf16, tag="ones2")
        nc.vector.memset(ones2, 1.0)

        hid_sb = singles.tile([64, H], f32, tag="hid_sb")
        sml_eng.dma_start(out=hid_sb, in_=hid)
        bias_sb = singles.tile([128, 8], f32, tag="bias_sb")
        sml_eng.dma_start(
            out=bias_sb,
            in_=bass.AP(tensor=bias.tensor, offset=bias.offset,
                        ap=[[1, 128], [128, 8]]))

        # hidT[k] : (128k, 64b) via PE transpose
        hidT = []
        for k in range(8):
            pt = psum.tile([128, 64], f32, tag="pp")
            nc.tensor.transpose(pt, hid_sb[:, k * 128:(k + 1) * 128], ident64)
            st = singles.tile([128, 64], f32, tag=f"hidT_{k}")
            nc.vector.tensor_copy(st, pt)
            hidT.append(st)

        # u = hid @ W : (64, H) via PE, accumulated over k in PSUM
        u_psum = psum1.tile([64, H], f32, tag="u_psum")
        for k in range(8):
            wk = wpool.tile([128, H], f32, tag="wk")
            sml_eng.dma_start(out=wk, in_=W[k * 128:(k + 1) * 128, :])
            for nh in range(2):
                nc.tensor.matmul(
                    u_psum[:, nh * 512:(nh + 1) * 512],
                    lhsT=hidT[k][:, 0:64],
                    rhs=wk[:, nh * 512:(nh + 1) * 512],
                    start=(k == 0), stop=(k == 7))

        # c_row = sum_k bias_k^T @ hidT_k : (1, 64);  cbF[p, b] = c[b]
        c_psum = psum.tile([1, 64], f32, tag="pp")
        for k in range(8):
            nc.tensor.matmul(c_psum, lhsT=bias_sb[:, k:k + 1], rhs=hidT[k],
                             start=(k == 0), stop=(k == 7))
        c_row = singles.tile([1, 64], f32, tag="c_row")
        nc.vector.tensor_copy(c_row, c_psum)
        cb_psum = psum.tile([128, 64], f32, tag="pp")
        nc.tensor.matmul(cb_psum, lhsT=ones1, rhs=c_row, start=True, stop=True)
        cbF = singles.tile([128, 64], f32, tag="cbF")
        nc.vector.tensor_copy(cbF, cb_psum)
        if not use_ttr:
            cbH = singles.tile([128, 64], f32, tag="cbH")
            nc.vector.tensor_scalar_mul(cbH, cb_psum, 1.0 / H)
        junk = singles.tile([128, 1], f32, tag="junk")

        # Split u into 2 exact bf16 terms (hi+mid carries ~17 mantissa bits;
        # the K=2 bf16 PE broadcast below reconstructs u to ~1e-5 accuracy).
        usplit = singles.tile([64, 2, H], bf16, tag="usplit")
        rtmp = singles.tile([64, H], f32, tag="rtmp")
        nc.vector.tensor_copy(usplit[:, 0, :], u_psum)
        nc.vector.tensor_sub(rtmp, u_psum, usplit[:, 0, :])
        nc.vector.tensor_copy(usplit[:, 1, :], rtmp)

        # ---------- phase 1: energies ----------
        # Er[r][p, b] = energy(b, s = 2p + r)
        Eh = [singles.tile([pp, B], f32, tag=f"E{i}", name=f"E{i}")
              for i in range(2)]

        for b in range(B):
            # ub[s, h] = u[b, h] broadcast into PSUM: stage the 2 bf16 split
            # rows of u[b] onto partitions 0-1 (tiny sync-ring DMA), then one
            # K=2 bf16 ones-matmul per 512-wide half sums hi+mid on all 128
            # partitions.
            if b % 2 == 0 and b // 2 + PF < NCHUNK:
                issue_ck(b // 2 + PF)
            stg = stgpool.tile([2, H], bf16, tag="stg")
            sml_eng.dma_start(out=stg, in_=usplit[b:b + 1, :, :])
            ub = psumB.tile([128, H], f32, tag="ub")
            for nh in range(2):
                nc.tensor.matmul(ub[:, nh * 512:(nh + 1) * 512],
                                 lhsT=ones2,
                                 rhs=stg[:, nh * 512:(nh + 1) * 512],
                                 start=True, stop=True)
            ck = cks[b // 2] if b < 2 * NPAIR else cks[NPAIR + (b - 2 * NPAIR)]
            b2 = (b % 2) if b < 2 * NPAIR else 0
            for r in range(2):
                if use_ttr == 1:
                    # ONE DVE pass: E[p,b] = c[b] + sum_h ck[p,r,h] * u[b,h]
                    nc.vector.tensor_tensor_reduce(
                        out=ck[:, b2, r, :],
                        in0=ck[:, b2, r, :],
                        in1=ub[0:pp, :],
                        scale=1.0,
                        scalar=cbF[0:pp, b:b + 1],
                        op0=Alu.mult,
                        op1=Alu.add,
                        accum_out=Eh[r][:, b:b + 1])
                elif use_ttr == 2:
                    # qr.py-style ttr: separate (junk) output, float initial;
                    # c[b] is added once per r after the loop.
                    nc.vector.tensor_tensor_reduce(
                        out=junk.broadcast_to(ck[:, b2, r, :].shape),
                        in0=ck[:, b2, r, :],
                        in1=ub[0:pp, :],
                        scale=1.0,
                        scalar=0.0,
                        op0=Alu.mult,
                        op1=Alu.add,
                        accum_out=Eh[r][:, b:b + 1])
                else:
                    nc.vector.tensor_mul(ck[:, b2, r, :], ck[:, b2, r, :],
                                         ub[0:pp, :])
                    nc.scalar.activation(ck[:, b2, r, :], ck[:, b2, r, :],
                                         Act.Identity,
                                         bias=cbH[0:pp, b:b + 1], scale=1.0,
                                         accum_out=Eh[r][:, b:b + 1])
        if use_ttr == 2:
            for r in range(2):
                nc.vector.tensor_add(Eh[r], Eh[r], cbF)

        # ---------- phase 2: softmax over b (free axis), emit out ----------
        O = small.tile([64, pp, 2], f32, tag="O")
        for r in range(2):
            e = Eh[r]
            negm = small.tile([pp, 1], f32, tag=f"negm{r}")
            nc.vector.tensor_reduce(negm, e, axis=X, op=Alu.max, negate=True)
            ssum = small.tile([pp, 1], f32, tag=f"ssum{r}")
            nc.scalar.activation(e, e, Act.Exp, bias=negm, scale=1.0,
                                 accum_out=ssum)
            rs = small.tile([pp, 1], f32, tag=f"rs{r}")
            nc.vector.reciprocal(rs, ssum)
            nc.vector.tensor_scalar_mul(e, e, rs)
            # transpose (pp s', 64b) -> (64b, pp s'), interleave r
            op = psum.tile([64, pp], f32, tag="pp")
            nc.tensor.transpose(op, e, ident128)
            nc.vector.tensor_copy(O[:, :, r], op)
        outv = out.rearrange("b (p r) -> b p r", r=2)
        nc.sync.dma_start(out=outv, in_=O)

    nc.compile()
    return nc


def _get_nc():
    if "nc" not in _CACHE:
        _CACHE["nc"] = build_nc()
    return _CACHE["nc"]


def run_spmd(hidden, encoder_output, W, b, **spmd_kwargs):
    from concourse.bass_utils import run_bass_kernel_spmd

    nc = _get_nc()
    hid2d = np.ascontiguousarray(np.asarray(hidden, dtype=np.float32)[0])
    Wn = np.ascontiguousarray(np.asarray(W, dtype=np.float32))
    bn = np.ascontiguousarray(np.asarray(b, dtype=np.float32).reshape(1, H))
    enc = np.asarray(encoder_output, dtype=np.float32)
    in_maps = []
    for c in range(N_CORES):
        in_maps.append({
            "enc": np.ascontiguousarray(enc[:, c * S_LOC:(c + 1) * S_LOC, :]),
            "hid": hid2d,
            "W": Wn,
            "bias": bn,
        })
    return run_bass_kernel_spmd(nc, in_maps, core_ids=list(range(N_CORES)),
                                **spmd_kwargs)


def kernel(hidden, encoder_output, W, b):
    res = run_spmd(hidden, encoder_output, W, b)
    return np.concatenate([res.results[c]["out"] for c in range(N_CORES)], axis=1)


# revision 16
# speedup vs baseline: 1.0746x; 1.0746x over previous
"""Trainium2 Bass kernel for nn_Attn_69776038691596.

reference computes:
    proj     = einsum('bsh,kh->bsk', enc, W) + bias          # (B,S,H)
    energies = einsum('bh,bsh->bs', hid, proj)               # (B,S)
    out      = softmax(energies, axis=0)                     # over batch

Algebraic rewrite (exact in real arithmetic):
    u[b,:] = hid[b,:] @ W          # (B,H)  -- tiny matmul
    c[b]   = hid[b,:] . bias       # (B,)
    energies[b,s] = enc[b,s,:] . u[b,:] + c[b]

This turns a 275-GFLOP matmul into a 0.27-GFLOP weighted reduction that is
bound by reading encoder_output (512 MB) from HBM once: 64 MiB/core at
~358 GB/s => ~188 us hard floor.

Sharding: split the S axis (2048 -> 8 x 256) across the 8 cores. The softmax
runs over the batch axis, which every core holds entirely, so no collectives
are needed.

Per-core schedule:
  - enc streams on the dedicated qActDynamicHW ring (nc.scalar) as 2 MB
    paired-b DMAs (8 KB contiguous per partition descriptor); W/hid/bias/out
    use the qSPDynamicHW ring (nc.sync); the tiny per-b stg staging DMAs use
    SWDGE (nc.gpsimd). A waiting DMA head-of-line-blocks only its own ring,
    so enc never stalls behind W/stg waits.
  - phase 0: u = hid @ W on PE (fp32, accumulated over 8 k-chunks in PSUM);
    u split into 2 exact bf16 terms (hi+mid carries ~17 mantissa bits);
    c_row = bias . hidT on PE; cbF[s,b] = c[b] broadcast via a K=1
    ones-matmul (used as the reduction's initial value).
  - main loop over b-pairs: one 2 MB DMA streams enc[2b:2b+2] into a
    (128, 2, 2, H) tile (partition p = s//2); u[b] is broadcast into PSUM by
    a K=2 bf16 ones-matmul; ONE DVE tensor_tensor_reduce per (b, r) does
    multiply + h-sum + c[b] offset in a single pass, writing E[s%2][s//2, b]
    directly (ScalarE is entirely free; DVE ~2.3us/b < DMA 2.9us/b).
  - softmax over the free (b) axis of each Er; PE-transpose the (128, 64b)
    results, DVE-interleave r, one output DMA.
"""
import sys

sys.path.insert(0, "/opt/trn_rl_repo")

import numpy as np

B, S, H = 64, 2048, 1024
N_CORES = 8
S_LOC = S // N_CORES  # 256

_CACHE = {}


def build_nc(s_loc=S_LOC, enc_ring=None, use_ttr=None):
    """Build + compile the per-core Bass module. s_loc must be divisible by 256."""
    import os
    if enc_ring is None:
        enc_ring = os.environ.get("K_ENC_RING", "sync")
    if use_ttr is None:
        use_ttr = int(os.environ.get("K_USE_TTR", "0"))
    small_ring = os.environ.get("K_SMALL_RING", "sync")
    import concourse.bass as bass
    import concourse.bacc as bacc
    import concourse.tile as tile
    from concourse import mybir
    from concourse.masks import make_identity
    from contextlib import ExitStack

    f32 = mybir.dt.float32
    bf16 = mybir.dt.bfloat16
    Alu = mybir.AluOpType
    Act = mybir.ActivationFunctionType
    X = mybir.AxisListType.X

    nc = bacc.Bacc("TRN2", target_bir_lowering=False, debug=False,
                   num_devices=N_CORES)
    enc = nc.dram_tensor("enc", [B, s_loc, H], f32, kind="ExternalInput").ap()
    hid = nc.dram_tensor("hid", [B, H], f32, kind="ExternalInput").ap()
    W = nc.dram_tensor("W", [H, H], f32, kind="ExternalInput").ap()
    bias = nc.dram_tensor("bias", [1, H], f32, kind="ExternalInput").ap()
    out = nc.dram_tensor("out", [B, s_loc], f32, kind="ExternalOutput").ap()

    pp = s_loc // 2  # 128 partitions, s = 2p + r

    with ExitStack() as ctx:
        tc = ctx.enter_context(tile.TileContext(nc))
        singles = ctx.enter_context(tc.tile_pool(name="singles", bufs=1))
        wpool = ctx.enter_context(tc.tile_pool(name="wpool", bufs=8))
        chunks = ctx.enter_context(tc.tile_pool(name="chunks", bufs=8))
        small = ctx.enter_context(tc.tile_pool(name="small", bufs=1))
        psum = ctx.enter_context(tc.tile_pool(name="psum", bufs=2, space="PSUM"))
        psum1 = ctx.enter_context(tc.tile_pool(name="psum1", bufs=1, space="PSUM"))
        psumB = ctx.enter_context(tc.tile_pool(name="psumB", bufs=2, space="PSUM"))
        stgpool = ctx.enter_context(tc.tile_pool(name="stgpool", bufs=6))
        prodp = ctx.enter_context(tc.tile_pool(name="prodp", bufs=3))

        # ---------- enc stream: dedicated scalar (ACT) HWDGE ring ----------
        # paired-b tiles: partition p = s//2, free (b2, r, h); descriptors are
        # 8 KB contiguous per (partition, b2).
        enc_eng = nc.scalar if enc_ring == "scalar" else nc.sync
        sml_eng = nc.scalar if small_ring == "scalar" else nc.sync
        enc1 = enc.rearrange("b (p r) h -> b p (r h)", r=2)
        NPAIR = B // 2 - 1  # last 2 b's go as singles to shorten the tail
        NCHUNK = NPAIR + 2
        cks = []

        def issue_ck(i):
            """Issue the i-th enc chunk DMA (pairs, then two singles)."""
            if i < NPAIR:
                ck = chunks.tile([pp, 2, 2, H], f32, tag="ck", name=f"ck{i}")
                pair_ap = bass.AP(
                    tensor=enc.tensor,
                    offset=enc.offset + i * 2 * s_loc * H,
                    ap=[[2 * H, pp], [s_loc * H, 2], [1, 2 * H]])
                enc_eng.dma_start(out=ck, in_=pair_ap)
            else:
                ck = chunks.tile([pp, 1, 2, H], f32, tag="ck", name=f"ck{i}")
                enc_eng.dma_start(out=ck[:, 0, :, :],
                                  in_=enc1[2 * NPAIR + (i - NPAIR)])
            cks.append(ck)

        # Prefetch fewer chunks than the pool holds so an enc DMA never waits
        # on a slot freed by compute that sits behind it in its own FIFO ring.
        # Ring order: ck0, hid, bias, W0..7, ck1..PF-1 -- phase-0 loads land
        # ~20us in while the enc stream keeps the SDMA engines saturated.
        PF = 7
        issue_ck(0)

        # ---------- phase 0 (sync ring + PE/DVE) ----------
        ident64 = singles.tile([64, 64], f32, tag="ident64")
        make_identity(nc, ident64)
        ident128 = singles.tile([128, 128], f32, tag="ident128")
        make_identity(nc, ident128)
        ones1 = singles.tile([1, 128], f32, tag="ones1")
        nc.vector.memset(ones1, 1.0)
        ones2 = singles.tile([2, 128], bf16, tag="ones2")
        nc.vector.memset(ones2, 1.0)

        hid_sb = singles.tile([64, H], f32, tag="hid_sb")
        sml_eng.dma_start(out=hid_sb, in_=hid)
        bias_sb = singles.tile([128, 8], f32, tag="bias_sb")
        sml_eng.dma_start(
            out=bias_sb,
            in_=bass.AP(tensor=bias.tensor, offset=bias.offset,
                        ap=[[1, 128], [128, 8]]))

        # hidT[k] : (128k, 64b) via PE transpose
        hidT = []
        for k in range(8):
            pt = psum.tile([128, 64], f32, tag="pp")
            nc.tensor.transpose(pt, hid_sb[:, k * 128:(k + 1) * 128], ident64)
            st = singles.tile([128, 64], f32, tag=f"hidT_{k}")
            nc.vector.tensor_copy(st, pt)
            hidT.append(st)

        # u = hid @ W : (64, H) via PE, accumulated over k in PSUM
        u_psum = psum1.tile([64, H], f32, tag="u_psum")
        for k in range(8):
            wk = wpool.tile([128, H], f32, tag="wk")
            sml_eng.dma_start(out=wk, in_=W[k * 128:(k + 1) * 128, :])
            for nh in range(2):
                nc.tensor.matmul(
                    u_psum[:, nh * 512:(nh + 1) * 512],
                    lhsT=hidT[k][:, 0:64],
                    rhs=wk[:, nh * 512:(nh + 1) * 512],
                    start=(k == 0), stop=(k == 7))

        for i in range(1, PF):
            issue_ck(i)

        # c_row = sum_k bias_k^T @ hidT_k : (1, 64);  cbH[p, b] = c[b]/H
        c_psum = psum.tile([1, 64], f32, tag="pp")
        for k in range(8):
            nc.tensor.matmul(c_psum, lhsT=bias_sb[:, k:k + 1], rhs=hidT[k],
                             start=(k == 0), stop=(k == 7))
        c_row = singles.tile([1, 64], f32, tag="c_row")
        nc.vector.tensor_copy(c_row, c_psum)
        cb_psum = psum.tile([128, 64], f32, tag="pp")
        nc.tensor.matmul(cb_psum, lhsT=ones1, rhs=c_row, start=True, stop=True)
        cbH = singles.tile([128, 64], f32, tag="cbH")
        nc.vector.tensor_scalar_mul(cbH, cb_psum, 1.0 / H)

        # Split u into 2 exact bf16 terms (hi+mid carries ~17 mantissa bits;
        # the K=2 bf16 PE broadcast below reconstructs u to ~1e-5 accuracy).
        usplit = singles.tile([64, 2, H], bf16, tag="usplit")
        rtmp = singles.tile([64, H], f32, tag="rtmp")
        nc.vector.tensor_copy(usplit[:, 0, :], u_psum)
        nc.vector.tensor_sub(rtmp, u_psum, usplit[:, 0, :])
        nc.vector.tensor_copy(usplit[:, 1, :], rtmp)

        # ---------- phase 1: energies ----------
        # Er[r][p, b] = energy(b, s = 2p + r)
        Eh = [singles.tile([pp, B], f32, tag=f"E{i}", name=f"E{i}")
              for i in range(2)]

        for b in range(B):
            # ub[s, h] = u[b, h] broadcast into PSUM: stage the 2 bf16 split
            # rows of u[b] onto partitions 0-1 (tiny sync-ring DMA), then one
            # K=2 bf16 ones-matmul per 512-wide half sums hi+mid on all 128
            # partitions.
            if b % 2 == 0 and b // 2 + PF < NCHUNK:
                issue_ck(b // 2 + PF)
            stg = stgpool.tile([2, H], bf16, tag="stg")
            sml_eng.dma_start(out=stg, in_=usplit[b:b + 1, :, :])
            ub = psumB.tile([128, H], f32, tag="ub")
            for nh in range(2):
                nc.tensor.matmul(ub[:, nh * 512:(nh + 1) * 512],
                                 lhsT=ones2,
                                 rhs=stg[:, nh * 512:(nh + 1) * 512],
                                 start=True, stop=True)
            ck = cks[b // 2] if b < 2 * NPAIR else cks[NPAIR + (b - 2 * NPAIR)]
            b2 = (b % 2) if b < 2 * NPAIR else 0
            for r in range(2):
                # DVE multiply into a rotating scratch tile (keeps the chunk
                # read-only so muls/accums pipeline across b and the chunk
                # slot frees as soon as its 4 muls have read it), then ACT
                # accumulates sum_h (+c[b]/H folded via the bias) into E.
                prod = prodp.tile([pp, H], f32, tag="prod")
                nc.vector.tensor_mul(prod, ck[:, b2, r, :], ub[0:pp, :])
                nc.scalar.activation(prod, prod, Act.Identity,
                                     bias=cbH[0:pp, b:b + 1], scale=1.0,
                                     accum_out=Eh[r][:, b:b + 1])

        # ---------- phase 2: softmax over b (free axis), emit out ----------
        O = small.tile([64, pp, 2], f32, tag="O")
        for r in range(2):
            e = Eh[r]
            negm = small.tile([pp, 1], f32, tag=f"negm{r}")
            nc.vector.tensor_reduce(negm, e, axis=X, op=Alu.max, negate=True)
            ssum = small.tile([pp, 1], f32, tag=f"ssum{r}")
            nc.scalar.activation(e, e, Act.Exp, bias=negm, scale=1.0,
                                 accum_out=ssum)
            rs = small.tile([pp, 1], f32, tag=f"rs{r}")
            nc.vector.reciprocal(rs, ssum)
            nc.vector.tensor_scalar_mul(e, e, rs)
            # transpose (pp s', 64b) -> (64b, pp s'), interleave r
            op = psum.tile([64, pp], f32, tag="pp")
            nc.tensor.transpose(op, e, ident128)
            nc.vector.tensor_copy(O[:, :, r], op)
        outv = out.rearrange("b (p r) -> b p r", r=2)
        nc.sync.dma_start(out=outv, in_=O)

    nc.compile()
    return nc


def _get_nc():
    if "nc" not in _CACHE:
        _CACHE["nc"] = build_nc()
    return _CACHE["nc"]


def run_spmd(hidden, encoder_output, W, b, **spmd_kwargs):
    from concourse.bass_utils import run_bass_kernel_spmd

    nc = _get_nc()
    hid2d = np.ascontiguousarray(np.asarray(hidden, dtype=np.float32)[0])
    Wn = np.ascontiguousarray(np.asarray(W, dtype=np.float32))
    bn = np.ascontiguousarray(np.asarray(b, dtype=np.float32).reshape(1, H))
    enc = np.asarray(encoder_output, dtype=np.float32)
    in_maps = []
    for c in range(N_CORES):
        in_maps.append({
            "enc": np.ascontiguousarray(enc[:, c * S_LOC:(c + 1) * S_LOC, :]),
            "hid": hid2d,
            "W": Wn,
            "bias": bn,
        })
    return run_bass_kernel_spmd(nc, in_maps, core_ids=list(range(N_CORES)),
                                **spmd_kwargs)


def kernel(hidden, encoder_output, W, b):
    res = run_spmd(hidden, encoder_output, W, b)
    return np.concatenate([res.results[c]["out"] for c in range(N_CORES)], axis=1)


# revision 17
# speedup vs baseline: 1.1200x; 1.0422x over previous
"""Trainium2 Bass kernel for nn_Attn_69776038691596.

reference computes:
    proj     = einsum('bsh,kh->bsk', enc, W) + bias          # (B,S,H)
    energies = einsum('bh,bsh->bs', hid, proj)               # (B,S)
    out      = softmax(energies, axis=0)                     # over batch

Algebraic rewrite (exact in real arithmetic):
    u[b,:] = hid[b,:] @ W          # (B,H)  -- tiny matmul
    c[b]   = hid[b,:] . bias       # (B,)
    energies[b,s] = enc[b,s,:] . u[b,:] + c[b]

This turns a 275-GFLOP matmul into a 0.27-GFLOP weighted reduction that is
bound by reading encoder_output (512 MB) from HBM once: 64 MiB/core at
~358 GB/s => ~188 us hard floor.

Sharding: split the S axis (2048 -> 8 x 256) across the 8 cores. The softmax
runs over the batch axis, which every core holds entirely, so no collectives
are needed.

Per-core schedule:
  - enc streams on the dedicated qActDynamicHW ring (nc.scalar) as 2 MB
    paired-b DMAs (8 KB contiguous per partition descriptor); W/hid/bias/out
    use the qSPDynamicHW ring (nc.sync); the tiny per-b stg staging DMAs use
    SWDGE (nc.gpsimd). A waiting DMA head-of-line-blocks only its own ring,
    so enc never stalls behind W/stg waits.
  - phase 0: u = hid @ W on PE (fp32, accumulated over 8 k-chunks in PSUM);
    u split into 2 exact bf16 terms (hi+mid carries ~17 mantissa bits);
    c_row = bias . hidT on PE; cbF[s,b] = c[b] broadcast via a K=1
    ones-matmul (used as the reduction's initial value).
  - main loop over b-pairs: one 2 MB DMA streams enc[2b:2b+2] into a
    (128, 2, 2, H) tile (partition p = s//2); u[b] is broadcast into PSUM by
    a K=2 bf16 ones-matmul; ONE DVE tensor_tensor_reduce per (b, r) does
    multiply + h-sum + c[b] offset in a single pass, writing E[s%2][s//2, b]
    directly (ScalarE is entirely free; DVE ~2.3us/b < DMA 2.9us/b).
  - softmax over the free (b) axis of each Er; PE-transpose the (128, 64b)
    results, DVE-interleave r, one output DMA.
"""
import sys

sys.path.insert(0, "/opt/trn_rl_repo")

import numpy as np

B, S, H = 64, 2048, 1024
N_CORES = 8
S_LOC = S // N_CORES  # 256

_CACHE = {}


def build_nc(s_loc=S_LOC, enc_ring=None, use_ttr=None):
    """Build + compile the per-core Bass module. s_loc must be divisible by 256."""
    import os
    if enc_ring is None:
        enc_ring = os.environ.get("K_ENC_RING", "sync")
    if use_ttr is None:
        use_ttr = int(os.environ.get("K_USE_TTR", "0"))
    small_ring = os.environ.get("K_SMALL_RING", "scalar")
    import concourse.bass as bass
    import concourse.bacc as bacc
    import concourse.tile as tile
    from concourse import mybir
    from concourse.masks import make_identity
    from contextlib import ExitStack

    f32 = mybir.dt.float32
    bf16 = mybir.dt.bfloat16
    Alu = mybir.AluOpType
    Act = mybir.ActivationFunctionType
    X = mybir.AxisListType.X

    nc = bacc.Bacc("TRN2", target_bir_lowering=False, debug=False,
                   num_devices=N_CORES)
    enc = nc.dram_tensor("enc", [B, s_loc, H], f32, kind="ExternalInput").ap()
    hid = nc.dram_tensor("hid", [B, H], f32, kind="ExternalInput").ap()
    W = nc.dram_tensor("W", [H, H], f32, kind="ExternalInput").ap()
    bias = nc.dram_tensor("bias", [1, H], f32, kind="ExternalInput").ap()
    out = nc.dram_tensor("out", [B, s_loc], f32, kind="ExternalOutput").ap()

    pp = s_loc // 2  # 128 partitions, s = 2p + r

    with ExitStack() as ctx:
        tc = ctx.enter_context(tile.TileContext(nc))
        singles = ctx.enter_context(tc.tile_pool(name="singles", bufs=1))
        wpool = ctx.enter_context(tc.tile_pool(name="wpool", bufs=2))
        chunks = ctx.enter_context(tc.tile_pool(name="chunks", bufs=9))
        small = ctx.enter_context(tc.tile_pool(name="small", bufs=1))
        psum = ctx.enter_context(tc.tile_pool(name="psum", bufs=2, space="PSUM"))
        psumB = ctx.enter_context(tc.tile_pool(name="psumB", bufs=3, space="PSUM"))
        stgpool = ctx.enter_context(tc.tile_pool(name="stgpool", bufs=6))
        prodp = ctx.enter_context(tc.tile_pool(name="prodp", bufs=2))

        # ---------- enc stream: dedicated scalar (ACT) HWDGE ring ----------
        # paired-b tiles: partition p = s//2, free (b2, r, h); descriptors are
        # 8 KB contiguous per (partition, b2).
        enc_eng = nc.scalar if enc_ring == "scalar" else nc.sync
        sml_eng = nc.scalar if small_ring == "scalar" else nc.sync
        enc1 = enc.rearrange("b (p r) h -> b p (r h)", r=2)
        NPAIR = B // 2 - 1  # last 2 b's go as singles to shorten the tail
        NCHUNK = NPAIR + 2
        cks = []

        def issue_ck(i):
            """Issue the i-th enc chunk DMA (pairs, then two singles)."""
            if i < NPAIR:
                ck = chunks.tile([pp, 2, 2, H], f32, tag="ck", name=f"ck{i}")
                pair_ap = bass.AP(
                    tensor=enc.tensor,
                    offset=enc.offset + i * 2 * s_loc * H,
                    ap=[[2 * H, pp], [s_loc * H, 2], [1, 2 * H]])
                enc_eng.dma_start(out=ck, in_=pair_ap)
            else:
                ck = chunks.tile([pp, 1, 2, H], f32, tag="ck", name=f"ck{i}")
                enc_eng.dma_start(out=ck[:, 0, :, :],
                                  in_=enc1[2 * NPAIR + (i - NPAIR)])
            cks.append(ck)

        # Prefetch fewer chunks than the pool holds so an enc DMA never waits
        # on a slot freed by compute that sits behind it in its own FIFO ring.
        # Ring order: ck0, hid, bias, W0..7, ck1..PF-1 -- phase-0 loads land
        # ~20us in while the enc stream keeps the SDMA engines saturated.
        PF = 7
        issue_ck(0)

        # ---------- phase 0 (sync ring + PE/DVE) ----------
        ident64 = singles.tile([64, 64], f32, tag="ident64")
        make_identity(nc, ident64)
        ident128 = singles.tile([128, 128], f32, tag="ident128")
        make_identity(nc, ident128)
        ones1 = singles.tile([1, 128], f32, tag="ones1")
        nc.vector.memset(ones1, 1.0)
        ones2 = singles.tile([2, 128], bf16, tag="ones2")
        nc.vector.memset(ones2, 1.0)

        hid_sb = singles.tile([64, H], f32, tag="hid_sb")
        sml_eng.dma_start(out=hid_sb, in_=hid)
        bias_sb = singles.tile([128, 8], f32, tag="bias_sb")
        sml_eng.dma_start(
            out=bias_sb,
            in_=bass.AP(tensor=bias.tensor, offset=bias.offset,
                        ap=[[1, 128], [128, 8]]))

        # hidT[k] : (128k, 64b) via PE transpose
        hidT = []
        for k in range(8):
            pt = psum.tile([128, 64], f32, tag="pp")
            nc.tensor.transpose(pt, hid_sb[:, k * 128:(k + 1) * 128], ident64)
            st = singles.tile([128, 64], f32, tag=f"hidT_{k}")
            nc.vector.tensor_copy(st, pt)
            hidT.append(st)

        # u = hid @ W : (64, H) via PE, accumulated over k in PSUM
        u_psum = psumB.tile([64, H], f32, tag="ub")
        for k in range(8):
            wk = wpool.tile([128, H], f32, tag="wk")
            sml_eng.dma_start(out=wk, in_=W[k * 128:(k + 1) * 128, :])
            for nh in range(2):
                nc.tensor.matmul(
                    u_psum[:, nh * 512:(nh + 1) * 512],
                    lhsT=hidT[k][:, 0:64],
                    rhs=wk[:, nh * 512:(nh + 1) * 512],
                    start=(k == 0), stop=(k == 7))

        for i in range(1, PF):
            issue_ck(i)

        # c_row = sum_k bias_k^T @ hidT_k : (1, 64);  cbH[p, b] = c[b]/H
        c_psum = psum.tile([1, 64], f32, tag="pp")
        for k in range(8):
            nc.tensor.matmul(c_psum, lhsT=bias_sb[:, k:k + 1], rhs=hidT[k],
                             start=(k == 0), stop=(k == 7))
        c_row = singles.tile([1, 64], f32, tag="c_row")
        nc.vector.tensor_copy(c_row, c_psum)
        cb_psum = psum.tile([128, 64], f32, tag="pp")
        nc.tensor.matmul(cb_psum, lhsT=ones1, rhs=c_row, start=True, stop=True)
        cbH = singles.tile([128, 64], f32, tag="cbH")
        nc.vector.tensor_scalar_mul(cbH, cb_psum, 1.0 / H)
        cbF = singles.tile([128, 64], f32, tag="cbF")
        nc.vector.tensor_copy(cbF, cb_psum)

        # Split u into 2 exact bf16 terms (hi+mid carries ~17 mantissa bits;
        # the K=2 bf16 PE broadcast below reconstructs u to ~1e-5 accuracy).
        usplit = singles.tile([64, 2, H], bf16, tag="usplit")
        rtmp = singles.tile([64, H], f32, tag="rtmp")
        nc.vector.tensor_copy(usplit[:, 0, :], u_psum)
        nc.vector.tensor_sub(rtmp, u_psum, usplit[:, 0, :])
        nc.vector.tensor_copy(usplit[:, 1, :], rtmp)

        # ---------- phase 1: energies ----------
        # Er[r][p, b] = energy(b, s = 2p + r)
        Eh = [singles.tile([pp, B], f32, tag=f"E{i}", name=f"E{i}")
              for i in range(2)]

        for b in range(B):
            # ub[s, h] = u[b, h] broadcast into PSUM: stage the 2 bf16 split
            # rows of u[b] onto partitions 0-1 (tiny sync-ring DMA), then one
            # K=2 bf16 ones-matmul per 512-wide half sums hi+mid on all 128
            # partitions.
            if b % 2 == 0 and b // 2 + PF < NCHUNK:
                issue_ck(b // 2 + PF)
            stg = stgpool.tile([2, H], bf16, tag="stg")
            sml_eng.dma_start(out=stg, in_=usplit[b:b + 1, :, :])
            ub = psumB.tile([128, H], f32, tag="ub")
            for nh in range(2):
                nc.tensor.matmul(ub[:, nh * 512:(nh + 1) * 512],
                                 lhsT=ones2,
                                 rhs=stg[:, nh * 512:(nh + 1) * 512],
                                 start=True, stop=True)
            ck = cks[b // 2] if b < 2 * NPAIR else cks[NPAIR + (b - 2 * NPAIR)]
            b2 = (b % 2) if b < 2 * NPAIR else 0
            # ONE fused DVE multiply per b over both r halves (2048 cols,
            # stride-0 broadcast of ub over r) into a rotating scratch; the
            # chunk stays read-only so the stream pipelines across b.
            prod = prodp.tile([pp, 2, H], f32, tag="prod")
            nc.vector.tensor_mul(
                prod, ck[:, b2, :, :],
                ub[0:pp, :].unsqueeze(1).broadcast_to((pp, 2, H)))
            # Reduction over h: ACT takes 3 of every 4 (bias folds c[b]/H);
            # DVE's spare capacity takes r=1 of even b (c added post-loop).
            nc.scalar.activation(prod[:, 0, :], prod[:, 0, :], Act.Identity,
                                 bias=cbH[0:pp, b:b + 1], scale=1.0,
                                 accum_out=Eh[0][:, b:b + 1])
            if b % 2 == 0:
                nc.vector.tensor_reduce(Eh[1][:, b:b + 1], prod[:, 1, :],
                                        axis=X, op=Alu.add)
            else:
                nc.scalar.activation(prod[:, 1, :], prod[:, 1, :],
                                     Act.Identity,
                                     bias=cbH[0:pp, b:b + 1], scale=1.0,
                                     accum_out=Eh[1][:, b:b + 1])

        # add the c[b] offset to the DVE-reduced (r=1, even b) columns
        Ev = Eh[1].rearrange("p (q two) -> p q two", two=2)
        cv = cbF.rearrange("p (q two) -> p q two", two=2)
        nc.vector.tensor_add(Ev[:, :, 0], Ev[:, :, 0], cv[:, :, 0])

        # ---------- phase 2: softmax over b (free axis), emit out ----------
        O = small.tile([64, pp, 2], f32, tag="O")
        for r in range(2):
            e = Eh[r]
            negm = small.tile([pp, 1], f32, tag=f"negm{r}")
            nc.vector.tensor_reduce(negm, e, axis=X, op=Alu.max, negate=True)
            ssum = small.tile([pp, 1], f32, tag=f"ssum{r}")
            nc.scalar.activation(e, e, Act.Exp, bias=negm, scale=1.0,
                                 accum_out=ssum)
            rs = small.tile([pp, 1], f32, tag=f"rs{r}")
            nc.vector.reciprocal(rs, ssum)
            nc.vector.tensor_scalar_mul(e, e, rs)
            # transpose (pp s', 64b) -> (64b, pp s'), interleave r
            op = psum.tile([64, pp], f32, tag="pp")
            nc.tensor.transpose(op, e, ident128)
            nc.vector.tensor_copy(O[:, :, r], op)
        outv = out.rearrange("b (p r) -> b p r", r=2)
        nc.sync.dma_start(out=outv, in_=O)

    nc.compile()
    return nc


def _get_nc():
    if "nc" not in _CACHE:
        _CACHE["nc"] = build_nc()
    return _CACHE["nc"]


def run_spmd(hidden, encoder_output, W, b, **spmd_kwargs):
    from concourse.bass_utils import run_bass_kernel_spmd

    nc = _get_nc()
    hid2d = np.ascontiguousarray(np.asarray(hidden, dtype=np.float32)[0])
    Wn = np.ascontiguousarray(np.asarray(W, dtype=np.float32))
    bn = np.ascontiguousarray(np.asarray(b, dtype=np.float32).reshape(1, H))
    enc = np.asarray(encoder_output, dtype=np.float32)
    in_maps = []
    for c in range(N_CORES):
        in_maps.append({
            "enc": np.ascontiguousarray(enc[:, c * S_LOC:(c + 1) * S_LOC, :]),
            "hid": hid2d,
            "W": Wn,
            "bias": bn,
        })
    return run_bass_kernel_spmd(nc, in_maps, core_ids=list(range(N_CORES)),
                                **spmd_kwargs)


def kernel(hidden, encoder_output, W, b):
    res = run_spmd(hidden, encoder_output, W, b)
    return np.concatenate([res.results[c]["out"] for c in range(N_CORES)], axis=1)


# revision 18
# speedup vs baseline: 1.1881x; 1.0609x over previous
"""Trainium2 Bass kernel for nn_Attn_69776038691596.

reference computes:
    proj     = einsum('bsh,kh->bsk', enc, W) + bias          # (B,S,H)
    energies = einsum('bh,bsh->bs', hid, proj)               # (B,S)
    out      = softmax(energies, axis=0)                     # over batch

Algebraic rewrite (exact in real arithmetic):
    u[b,:] = hid[b,:] @ W          # (B,H)  -- tiny matmul
    c[b]   = hid[b,:] . bias       # (B,)
    energies[b,s] = enc[b,s,:] . u[b,:] + c[b]

This turns a 275-GFLOP matmul into a 0.27-GFLOP weighted reduction that is
bound by reading encoder_output (512 MB) from HBM once: 64 MiB/core at
~358 GB/s => ~188 us hard floor.

Sharding: split the S axis (2048 -> 8 x 256) across the 8 cores. The softmax
runs over the batch axis, which every core holds entirely, so no collectives
are needed.

Per-core schedule:
  - enc streams on the dedicated qActDynamicHW ring (nc.scalar) as 2 MB
    paired-b DMAs (8 KB contiguous per partition descriptor); W/hid/bias/out
    use the qSPDynamicHW ring (nc.sync); the tiny per-b stg staging DMAs use
    SWDGE (nc.gpsimd). A waiting DMA head-of-line-blocks only its own ring,
    so enc never stalls behind W/stg waits.
  - phase 0: u = hid @ W on PE (fp32, accumulated over 8 k-chunks in PSUM);
    u split into 2 exact bf16 terms (hi+mid carries ~17 mantissa bits);
    c_row = bias . hidT on PE; cbF[s,b] = c[b] broadcast via a K=1
    ones-matmul (used as the reduction's initial value).
  - main loop over b-pairs: one 2 MB DMA streams enc[2b:2b+2] into a
    (128, 2, 2, H) tile (partition p = s//2); u[b] is broadcast into PSUM by
    a K=2 bf16 ones-matmul; ONE DVE tensor_tensor_reduce per (b, r) does
    multiply + h-sum + c[b] offset in a single pass, writing E[s%2][s//2, b]
    directly (ScalarE is entirely free; DVE ~2.3us/b < DMA 2.9us/b).
  - softmax over the free (b) axis of each Er; PE-transpose the (128, 64b)
    results, DVE-interleave r, one output DMA.
"""
import sys

sys.path.insert(0, "/opt/trn_rl_repo")

import numpy as np

B, S, H = 64, 2048, 1024
N_CORES = 8
S_LOC = S // N_CORES  # 256

_CACHE = {}


def build_nc(s_loc=S_LOC, enc_ring=None, use_ttr=None):
    """Build + compile the per-core Bass module. s_loc must be divisible by 256."""
    import os
    if enc_ring is None:
        enc_ring = os.environ.get("K_ENC_RING", "sync")
    if use_ttr is None:
        use_ttr = int(os.environ.get("K_USE_TTR", "0"))
    small_ring = os.environ.get("K_SMALL_RING", "scalar")
    import concourse.bass as bass
    import concourse.bacc as bacc
    import concourse.tile as tile
    from concourse import mybir
    from concourse.masks import make_identity
    from contextlib import ExitStack

    f32 = mybir.dt.float32
    bf16 = mybir.dt.bfloat16
    Alu = mybir.AluOpType
    Act = mybir.ActivationFunctionType
    X = mybir.AxisListType.X

    nc = bacc.Bacc("TRN2", target_bir_lowering=False, debug=False,
                   num_devices=N_CORES)
    enc = nc.dram_tensor("enc", [B, s_loc, H], f32, kind="ExternalInput").ap()
    hid = nc.dram_tensor("hid", [B, H], f32, kind="ExternalInput").ap()
    W = nc.dram_tensor("W", [H, H], f32, kind="ExternalInput").ap()
    bias = nc.dram_tensor("bias", [1, H], f32, kind="ExternalInput").ap()
    out = nc.dram_tensor("out", [B, s_loc], f32, kind="ExternalOutput").ap()

    pp = s_loc // 2  # 128 partitions, s = 2p + r

    with ExitStack() as ctx:
        tc = ctx.enter_context(tile.TileContext(nc))
        singles = ctx.enter_context(tc.tile_pool(name="singles", bufs=1))
        wpool = ctx.enter_context(tc.tile_pool(name="wpool", bufs=4))
        chunks = ctx.enter_context(tc.tile_pool(name="chunks", bufs=8))
        small = ctx.enter_context(tc.tile_pool(name="small", bufs=1))
        psum = ctx.enter_context(tc.tile_pool(name="psum", bufs=2, space="PSUM"))
        psumB = ctx.enter_context(tc.tile_pool(name="psumB", bufs=3, space="PSUM"))
        stgpool = ctx.enter_context(tc.tile_pool(name="stgpool", bufs=6))
        prodp = ctx.enter_context(tc.tile_pool(name="prodp", bufs=2))

        # ---------- enc stream: dedicated scalar (ACT) HWDGE ring ----------
        # paired-b tiles: partition p = s//2, free (b2, r, h); descriptors are
        # 8 KB contiguous per (partition, b2).
        enc_eng = nc.scalar if enc_ring == "scalar" else nc.sync
        sml_eng = nc.scalar if small_ring == "scalar" else nc.sync
        enc1 = enc.rearrange("b (p r) h -> b p (r h)", r=2)
        NPAIR = B // 2 - 1  # last 2 b's go as singles to shorten the tail
        NCHUNK = NPAIR + 2
        cks = []

        def issue_ck(i):
            """Issue the i-th enc chunk DMA (pairs, then two singles)."""
            if i < NPAIR:
                ck = chunks.tile([pp, 2, 2, H], f32, tag="ck", name=f"ck{i}")
                pair_ap = bass.AP(
                    tensor=enc.tensor,
                    offset=enc.offset + i * 2 * s_loc * H,
                    ap=[[2 * H, pp], [s_loc * H, 2], [1, 2 * H]])
                enc_eng.dma_start(out=ck, in_=pair_ap)
            else:
                ck = chunks.tile([pp, 1, 2, H], f32, tag="ck", name=f"ck{i}")
                enc_eng.dma_start(out=ck[:, 0, :, :],
                                  in_=enc1[2 * NPAIR + (i - NPAIR)])
            cks.append(ck)

        # Prefetch fewer chunks than the pool holds so an enc DMA never waits
        # on a slot freed by compute that sits behind it in its own FIFO ring.
        # Ring order: ck0, hid, bias, W0..7, ck1..PF-1 -- phase-0 loads land
        # ~20us in while the enc stream keeps the SDMA engines saturated.
        PF = 7
        issue_ck(0)

        # ---------- phase 0 (sync ring + PE/DVE) ----------
        ident64 = singles.tile([64, 64], f32, tag="ident64")
        make_identity(nc, ident64)
        ident128 = singles.tile([128, 128], f32, tag="ident128")
        make_identity(nc, ident128)
        ones1 = singles.tile([1, 128], f32, tag="ones1")
        nc.vector.memset(ones1, 1.0)
        ones2 = singles.tile([2, 128], bf16, tag="ones2")
        nc.vector.memset(ones2, 1.0)

        hid_sb = singles.tile([64, H], f32, tag="hid_sb")
        sml_eng.dma_start(out=hid_sb, in_=hid)
        bias_sb = singles.tile([128, 8], f32, tag="bias_sb")
        sml_eng.dma_start(
            out=bias_sb,
            in_=bass.AP(tensor=bias.tensor, offset=bias.offset,
                        ap=[[1, 128], [128, 8]]))

        # hidT[k] : (128k, 64b) via PE transpose
        hidT = []
        for k in range(8):
            pt = psum.tile([128, 64], f32, tag="pp")
            nc.tensor.transpose(pt, hid_sb[:, k * 128:(k + 1) * 128], ident64)
            st = singles.tile([128, 64], f32, tag=f"hidT_{k}")
            nc.vector.tensor_copy(st, pt)
            hidT.append(st)

        # u = hid @ W : (64, H) via PE, accumulated over k in PSUM.
        # W comes in as 4 unchained 1 MB DMAs (k-pairs) so the loads stream
        # back-to-back instead of serializing behind PE consumption.
        wks = []
        for j in range(4):
            wk = wpool.tile([128, 2, H], f32, tag="wk")
            sml_eng.dma_start(out=wk, in_=W[2 * j * 128:(2 * j + 2) * 128, :])
            wks.append(wk)
        u_psum = psumB.tile([64, H], f32, tag="ub")
        for k in range(8):
            for nh in range(2):
                nc.tensor.matmul(
                    u_psum[:, nh * 512:(nh + 1) * 512],
                    lhsT=hidT[k][:, 0:64],
                    rhs=wks[k // 2][:, k % 2, nh * 512:(nh + 1) * 512],
                    start=(k == 0), stop=(k == 7))

        for i in range(1, PF):
            issue_ck(i)

        # c_row = sum_k bias_k^T @ hidT_k : (1, 64);  cbH[p, b] = c[b]/H
        c_psum = psum.tile([1, 64], f32, tag="pp")
        for k in range(8):
            nc.tensor.matmul(c_psum, lhsT=bias_sb[:, k:k + 1], rhs=hidT[k],
                             start=(k == 0), stop=(k == 7))
        c_row = singles.tile([1, 64], f32, tag="c_row")
        nc.vector.tensor_copy(c_row, c_psum)
        cb_psum = psum.tile([128, 64], f32, tag="pp")
        nc.tensor.matmul(cb_psum, lhsT=ones1, rhs=c_row, start=True, stop=True)
        cbH = singles.tile([128, 64], f32, tag="cbH")
        nc.vector.tensor_scalar_mul(cbH, cb_psum, 1.0 / H)

        # Split u into 2 exact bf16 terms (hi+mid carries ~17 mantissa bits;
        # the K=2 bf16 PE broadcast below reconstructs u to ~1e-5 accuracy).
        usplit = singles.tile([64, 2, H], bf16, tag="usplit")
        rtmp = singles.tile([64, H], f32, tag="rtmp")
        nc.vector.tensor_copy(usplit[:, 0, :], u_psum)
        nc.vector.tensor_sub(rtmp, u_psum, usplit[:, 0, :])
        nc.vector.tensor_copy(usplit[:, 1, :], rtmp)

        # ---------- phase 1: energies ----------
        # Er[r][p, b] = energy(b, s = 2p + r)
        Eh = [singles.tile([pp, B], f32, tag=f"E{i}", name=f"E{i}")
              for i in range(2)]

        for b in range(B):
            # ub[s, h] = u[b, h] broadcast into PSUM: stage the 2 bf16 split
            # rows of u[b] onto partitions 0-1 (tiny sync-ring DMA), then one
            # K=2 bf16 ones-matmul per 512-wide half sums hi+mid on all 128
            # partitions.
            if b % 2 == 0 and b // 2 + PF < NCHUNK:
                issue_ck(b // 2 + PF)
            stg = stgpool.tile([2, H], bf16, tag="stg")
            sml_eng.dma_start(out=stg, in_=usplit[b:b + 1, :, :])
            ub = psumB.tile([128, H], f32, tag="ub")
            for nh in range(2):
                nc.tensor.matmul(ub[:, nh * 512:(nh + 1) * 512],
                                 lhsT=ones2,
                                 rhs=stg[:, nh * 512:(nh + 1) * 512],
                                 start=True, stop=True)
            ck = cks[b // 2] if b < 2 * NPAIR else cks[NPAIR + (b - 2 * NPAIR)]
            b2 = (b % 2) if b < 2 * NPAIR else 0
            # ONE fused DVE multiply per b over both r halves (2048 cols,
            # stride-0 broadcast of ub over r) into a rotating scratch; the
            # chunk stays read-only so the stream pipelines across b.
            prod = prodp.tile([pp, 2, H], f32, tag="prod")
            nc.vector.tensor_mul(
                prod, ck[:, b2, :, :],
                ub[0:pp, :].unsqueeze(1).broadcast_to((pp, 2, H)))
            # Reduction over h on ACT (bias folds in c[b]/H); DVE stays
            # mul-only so both engines sit ~1us/pair under the DMA pace.
            for r in range(2):
                nc.scalar.activation(prod[:, r, :], prod[:, r, :],
                                     Act.Identity,
                                     bias=cbH[0:pp, b:b + 1], scale=1.0,
                                     accum_out=Eh[r][:, b:b + 1])

        # ---------- phase 2: softmax over b (free axis), emit out ----------
        O = small.tile([64, pp, 2], f32, tag="O")
        for r in range(2):
            e = Eh[r]
            negm = small.tile([pp, 1], f32, tag=f"negm{r}")
            nc.vector.tensor_reduce(negm, e, axis=X, op=Alu.max, negate=True)
            ssum = small.tile([pp, 1], f32, tag=f"ssum{r}")
            nc.scalar.activation(e, e, Act.Exp, bias=negm, scale=1.0,
                                 accum_out=ssum)
            rs = small.tile([pp, 1], f32, tag=f"rs{r}")
            nc.vector.reciprocal(rs, ssum)
            nc.vector.tensor_scalar_mul(e, e, rs)
            # transpose (pp s', 64b) -> (64b, pp s'), interleave r
            op = psum.tile([64, pp], f32, tag="pp")
            nc.tensor.transpose(op, e, ident128)
            nc.vector.tensor_copy(O[:, :, r], op)
        outv = out.rearrange("b (p r) -> b p r", r=2)
        nc.sync.dma_start(out=outv, in_=O)

    nc.compile()
    return nc


def _get_nc():
    if "nc" not in _CACHE:
        _CACHE["nc"] = build_nc()
    return _CACHE["nc"]


def run_spmd(hidden, encoder_output, W, b, **spmd_kwargs):
    from concourse.bass_utils import run_bass_kernel_spmd

    nc = _get_nc()
    hid2d = np.ascontiguousarray(np.asarray(hidden, dtype=np.float32)[0])
    Wn = np.ascontiguousarray(np.asarray(W, dtype=np.float32))
    bn = np.ascontiguousarray(np.asarray(b, dtype=np.float32).reshape(1, H))
    enc = np.asarray(encoder_output, dtype=np.float32)
    in_maps = []
    for c in range(N_CORES):
        in_maps.append({
            "enc": np.ascontiguousarray(enc[:, c * S_LOC:(c + 1) * S_LOC, :]),
            "hid": hid2d,
            "W": Wn,
            "bias": bn,
        })
    return run_bass_kernel_spmd(nc, in_maps, core_ids=list(range(N_CORES)),
                                **spmd_kwargs)


def kernel(hidden, encoder_output, W, b):
    res = run_spmd(hidden, encoder_output, W, b)
    return np.concatenate([res.results[c]["out"] for c in range(N_CORES)], axis=1)
